# revision 1
# baseline (speedup 1.0000x reference)
"""Trainium2 Bass kernel for InvSGSS quantized linear.

out[m, k] = sum_n x[m, n] * W_deq[k, n] + bias[k]
W_deq[k, n] = (W_q[k, n] - zeros[k, g]) * scales[k, g] * mu2[k] * mu1[n],  g = n // 128

Sharding (8 cores): 2 m-shards x 4 k-shards. Each core handles
M_C=4096 rows of x and K_C=1024 output features.

Per-core dataflow:
  Phase 1 (once): DMA W_q shard [128k, N] int32 tiles; dequant on DVE with
    fused tensor_scalar (W*s' + b') where s' = scales*mu2, b' = -zeros*scales*mu2
    (host-folded small tensors); PE-transpose 128x128 chunks to build the
    resident W.T [n, k] bf16 operand, folding mu1[n] in during PSUM evict.
  Phase 2 (streamed): DMA x tiles [128m, N] with fp32->bf16 cast (SWDGE),
    PE-transpose to x.T [n-chunk, m] tiles, then 32 accumulating bf16 matmuls
    per (m-tile, k-tile) psum; bias added on psum evict.
"""

import sys

if "/opt/trn_rl_repo" not in sys.path:
    sys.path.insert(0, "/opt/trn_rl_repo")

import numpy as np

import concourse.bass as bass  # noqa: F401
import concourse.mybir as mybir
import concourse.tile as tile
from concourse import bacc
from concourse.bass_utils import run_bass_kernel_spmd
from concourse.masks import make_identity

K, N = 4096, 4096
GROUP = 128
NG = N // GROUP  # 32 groups along N (group == 128-chunk)
M = 8192  # B*S
B, S = 4, 2048
M_SH, K_SH = 2, 4  # core grid: 2 m-shards x 4 k-shards
MC = M // M_SH  # 4096 rows per core
KC = K // K_SH  # 1024 output features per core
NCH = N // 128  # 32 contraction chunks
MT = MC // 128  # 32 m-tiles
KT = KC // 128  # 8 k-row-tiles of W
KTILE = 512  # matmul free dim (one PSUM bank)
NKT = KC // KTILE  # 2

_CACHE: dict = {}


def build_nc(
    repeat: int = 1,
    debug: bool = False,
    x_cast: str = "act",
    probe: str = "full",
    xT_dma: bool = True,
):
    """x_cast: 'dma' = SWDGE cast-DMA fp32->bf16; 'act' = HWDGE fp32 DMA + ScalarE cast;
    'vec' = HWDGE fp32 DMA + VectorE cast.
    probe: 'full' | 'mm_only' (skip x load/transpose in repeat body) |
    'xprep_only' (skip matmuls in repeat body).
    xT_dma: transpose x tiles via xbar DMA instead of the PE."""
    dt = mybir.dt
    nc = bacc.Bacc("TRN2", target_bir_lowering=False, debug=debug)

    x_d = nc.dram_tensor("x", [MC, N], dt.float32, kind="ExternalInput")
    wq_d = nc.dram_tensor("wq", [KC, N], dt.int32, kind="ExternalInput")
    seff_d = nc.dram_tensor("seff", [KC, NG], dt.float32, kind="ExternalInput")
    beff_d = nc.dram_tensor("beff", [KC, NG], dt.float32, kind="ExternalInput")
    mu1_d = nc.dram_tensor("mu1t", [128, NG], dt.float32, kind="ExternalInput")
    bias_d = nc.dram_tensor("biasb", [128, KC], dt.float32, kind="ExternalInput")
    out_d = nc.dram_tensor("out", [MC, KC], dt.float32, kind="ExternalOutput")

    with tile.TileContext(nc) as tc:
        with tc.tile_pool(name="const", bufs=1) as cpool:
            ident = cpool.tile([128, 128], dt.bfloat16)
            make_identity(nc, ident)
            mu1_sb = cpool.tile([128, NG], dt.float32)
            nc.sync.dma_start(out=mu1_sb, in_=mu1_d[:, :])
            bias_sb = cpool.tile([128, KC], dt.float32)
            nc.sync.dma_start(out=bias_sb, in_=bias_d[:, :])
            seff_sb = cpool.tile([128, KT, NG], dt.float32)
            nc.sync.dma_start(
                out=seff_sb, in_=seff_d.rearrange("(t p) g -> p t g", p=128)
            )
            beff_sb = cpool.tile([128, KT, NG], dt.float32)
            nc.sync.dma_start(
                out=beff_sb, in_=beff_d.rearrange("(t p) g -> p t g", p=128)
            )

            # Resident transposed weight operand: [n % 128, n // 128, k]
            wt_sb = cpool.tile([128, NCH, KC], dt.bfloat16)

            # ---------------- Phase 1: dequant + transpose W ----------------
            with (
                tc.tile_pool(name="wq_pool", bufs=2) as wq_pool,
                tc.tile_pool(name="wstage", bufs=3) as ws_pool,
                tc.tile_pool(name="psw", bufs=2, space="PSUM") as psw_pool,
            ):
                for half in range(2):
                    wq_tiles = []
                    for i in range(4):
                        kt = half * 4 + i
                        wq_t = wq_pool.tile([128, N], dt.int32, name=f"wq_{i}")
                        nc.sync.dma_start(
                            out=wq_t, in_=wq_d[kt * 128 : (kt + 1) * 128, :]
                        )
                        wq_tiles.append((kt, wq_t))
                    for g in range(NG):
                        stage = ws_pool.tile([128, 4, 128], dt.bfloat16, name="wstg")
                        for i, (kt, wq_t) in enumerate(wq_tiles):
                            # (Q * s') + b'  with s' = scales*mu2, b' = -z*s*mu2
                            nc.vector.tensor_scalar(
                                out=stage[:, i, :],
                                in0=wq_t[:, g * 128 : (g + 1) * 128],
                                scalar1=seff_sb[:, kt, g : g + 1],
                                scalar2=beff_sb[:, kt, g : g + 1],
                                op0=mybir.AluOpType.mult,
                                op1=mybir.AluOpType.add,
                            )
                        ps = psw_pool.tile([128, 512], dt.bfloat16, name="psw")
                        for i in range(4):
                            nc.tensor.transpose(
                                ps[:, i * 128 : (i + 1) * 128], stage[:, i, :], ident
                            )
                        # evict with mu1[n] fold (per-partition scalar)
                        nc.vector.tensor_scalar_mul(
                            out=wt_sb[:, g, half * 512 : (half + 1) * 512],
                            in0=ps,
                            scalar1=mu1_sb[:, g : g + 1],
                        )

            # ---------------- Phase 2: stream x, matmul ----------------
            with (
                tc.tile_pool(name="xload", bufs=3) as xl_pool,
                tc.tile_pool(name="xt", bufs=2) as xt_pool,
                tc.tile_pool(name="psx", bufs=2, space="PSUM") as psx_pool,
                tc.tile_pool(name="pso", bufs=4, space="PSUM") as pso_pool,
                tc.tile_pool(name="osb", bufs=4) as osb_pool,
            ):
                def x_prep(mt, pool_tag=""):
                    xb = xl_pool.tile([128, N], dt.bfloat16, name="xb" + pool_tag)
                    if x_cast == "dma":
                        # SWDGE cast DMA: fp32 DRAM -> bf16 SBUF
                        nc.gpsimd.dma_start(
                            out=xb, in_=x_d[mt * 128 : (mt + 1) * 128, :]
                        )
                    else:
                        xf = xl_pool.tile([128, N], dt.float32, name="xf" + pool_tag)
                        nc.sync.dma_start(
                            out=xf, in_=x_d[mt * 128 : (mt + 1) * 128, :]
                        )
                        if x_cast == "act":
                            nc.scalar.copy(out=xb, in_=xf)
                        else:
                            nc.vector.tensor_copy(out=xb, in_=xf)
                    xt_t = xt_pool.tile([128, NCH, 128], dt.bfloat16, name="xt" + pool_tag)
                    if xT_dma:
                        # xbar DMA transpose SBUF->SBUF: [128 m, 4096 n] -> [n%128, n//128, m]
                        nc.scalar.dma_start(out=xt_t[:, :, :], in_=xb[:, :], transpose=True)
                    else:
                        for gb in range(NCH // 4):
                            ps = psx_pool.tile([128, 512], dt.bfloat16, name="psx")
                            for i in range(4):
                                g = gb * 4 + i
                                nc.tensor.transpose(
                                    ps[:, i * 128 : (i + 1) * 128],
                                    xb[:, g * 128 : (g + 1) * 128],
                                    ident,
                                )
                            nc.scalar.copy(
                                out=xt_t[:, gb * 4 : (gb + 1) * 4, :], in_=ps
                            )
                    return xt_t

                xt_fixed = x_prep(0, pool_tag="fix") if probe == "mm_only" else None
                for _rep in range(repeat):
                    for mt in range(MT):
                        if probe == "mm_only":
                            xt_t = xt_fixed
                        else:
                            xt_t = x_prep(mt)
                        if probe == "xprep_only":
                            continue
                        for kt2 in range(NKT):
                            pso = pso_pool.tile([128, KTILE], dt.float32, name="pso")
                            for g in range(NCH):
                                nc.tensor.matmul(
                                    pso,
                                    lhsT=xt_t[:, g, :],
                                    rhs=wt_sb[:, g, kt2 * KTILE : (kt2 + 1) * KTILE],
                                    start=(g == 0),
                                    stop=(g == NCH - 1),
                                )
                            osb = osb_pool.tile([128, KTILE], dt.float32, name="osb")
                            nc.vector.tensor_add(
                                out=osb,
                                in0=pso,
                                in1=bias_sb[:, kt2 * KTILE : (kt2 + 1) * KTILE],
                            )
                            nc.sync.dma_start(
                                out=out_d[
                                    mt * 128 : (mt + 1) * 128,
                                    kt2 * KTILE : (kt2 + 1) * KTILE,
                                ],
                                in_=osb,
                            )
    nc.compile()
    return nc


def make_in_maps(x, W_q, scales, zeros, mu1, mu2, bias):
    x2 = np.ascontiguousarray(np.asarray(x, dtype=np.float32).reshape(M, N))
    W_q = np.asarray(W_q, dtype=np.int32)
    scales = np.asarray(scales, dtype=np.float32).reshape(K, NG)
    zeros = np.asarray(zeros, dtype=np.float32).reshape(K, NG)
    mu1 = np.asarray(mu1, dtype=np.float32)
    mu2 = np.asarray(mu2, dtype=np.float32)
    bias = np.asarray(bias, dtype=np.float32)

    s_eff = scales * mu2[:, None]  # [K, NG]
    b_eff = -(zeros * s_eff)  # [K, NG]
    mu1_t = np.ascontiguousarray(mu1.reshape(NG, 128).T)  # [128, NG]

    in_maps = []
    for c in range(8):
        mi, ki = c // K_SH, c % K_SH
        in_maps.append(
            {
                "x": x2[mi * MC : (mi + 1) * MC],
                "wq": np.ascontiguousarray(W_q[ki * KC : (ki + 1) * KC]),
                "seff": np.ascontiguousarray(s_eff[ki * KC : (ki + 1) * KC]),
                "beff": np.ascontiguousarray(b_eff[ki * KC : (ki + 1) * KC]),
                "mu1t": mu1_t,
                "biasb": np.ascontiguousarray(
                    np.broadcast_to(bias[ki * KC : (ki + 1) * KC], (128, KC))
                ),
            }
        )
    return in_maps


def assemble(results):
    out = np.empty((M, K), np.float32)
    for c in range(8):
        mi, ki = c // K_SH, c % K_SH
        out[mi * MC : (mi + 1) * MC, ki * KC : (ki + 1) * KC] = results[c]["out"]
    return out.reshape(B, S, K)


def kernel(x, W_q, scales, zeros, mu1, mu2, bias):
    in_maps = make_in_maps(x, W_q, scales, zeros, mu1, mu2, bias)
    nc = _CACHE.get("nc")
    if nc is None:
        nc = build_nc()
        _CACHE["nc"] = nc
    res = run_bass_kernel_spmd(nc, in_maps, core_ids=list(range(8)))
    return assemble(res.results)



# revision 2
# speedup vs baseline: 1.0930x; 1.0930x over previous
"""Trainium2 Bass kernel for InvSGSS quantized linear.

out[m, k] = sum_n x[m, n] * W_deq[k, n] + bias[k]
W_deq[k, n] = (W_q[k, n] - zeros[k, g]) * scales[k, g] * mu2[k] * mu1[n],  g = n // 128

Sharding (8 cores): 2 m-shards x 4 k-shards. Each core handles
M_C=4096 rows of x and K_C=1024 output features.

Per-core dataflow:
  Phase 1 (once): DMA W_q shard [128k, N] int32 tiles; dequant on DVE with
    fused tensor_scalar (W*s' + b') where s' = scales*mu2, b' = -zeros*scales*mu2
    (host-folded small tensors); PE-transpose 128x128 chunks to build the
    resident W.T [n, k] bf16 operand, folding mu1[n] in during PSUM evict.
  Phase 2 (streamed): DMA x tiles [128m, N] with fp32->bf16 cast (SWDGE),
    PE-transpose to x.T [n-chunk, m] tiles, then 32 accumulating bf16 matmuls
    per (m-tile, k-tile) psum; bias added on psum evict.
"""

import sys

if "/opt/trn_rl_repo" not in sys.path:
    sys.path.insert(0, "/opt/trn_rl_repo")

import numpy as np

import concourse.bass as bass  # noqa: F401
import concourse.mybir as mybir
import concourse.tile as tile
from concourse import bacc
from concourse.bass_utils import run_bass_kernel_spmd
from concourse.masks import make_identity

K, N = 4096, 4096
GROUP = 128
NG = N // GROUP  # 32 groups along N (group == 128-chunk)
M = 8192  # B*S
B, S = 4, 2048
M_SH, K_SH = 2, 4  # core grid: 2 m-shards x 4 k-shards
MC = M // M_SH  # 4096 rows per core
KC = K // K_SH  # 1024 output features per core
NCH = N // 128  # 32 contraction chunks
MT = MC // 128  # 32 m-tiles
KT = KC // 128  # 8 k-row-tiles of W
KTILE = 512  # matmul free dim (one PSUM bank)
NKT = KC // KTILE  # 2

_CACHE: dict = {}


def build_nc(
    repeat: int = 1,
    debug: bool = False,
    x_cast: str = "dma",
    probe: str = "full",
    xT_dma: bool = True,
):
    """x_cast: 'dma' = SWDGE cast-DMA fp32->bf16; 'act' = HWDGE fp32 DMA + ScalarE cast;
    'vec' = HWDGE fp32 DMA + VectorE cast.
    probe: 'full' | 'mm_only' (skip x load/transpose in repeat body) |
    'xprep_only' (skip matmuls in repeat body).
    xT_dma: transpose x tiles via xbar DMA instead of the PE."""
    dt = mybir.dt
    nc = bacc.Bacc("TRN2", target_bir_lowering=False, debug=debug)

    x_d = nc.dram_tensor("x", [MC, N], dt.float32, kind="ExternalInput")
    wq_d = nc.dram_tensor("wq", [KC, N], dt.int32, kind="ExternalInput")
    seff_d = nc.dram_tensor("seff", [KC, NG], dt.float32, kind="ExternalInput")
    beff_d = nc.dram_tensor("beff", [KC, NG], dt.float32, kind="ExternalInput")
    mu1_d = nc.dram_tensor("mu1t", [128, NG], dt.float32, kind="ExternalInput")
    bias_d = nc.dram_tensor("biasb", [128, KC], dt.float32, kind="ExternalInput")
    out_d = nc.dram_tensor("out", [MC, KC], dt.float32, kind="ExternalOutput")

    with tile.TileContext(nc) as tc:
        with tc.tile_pool(name="const", bufs=1) as cpool:
            ident = cpool.tile([128, 128], dt.bfloat16)
            make_identity(nc, ident)
            mu1_sb = cpool.tile([128, NG], dt.float32)
            nc.sync.dma_start(out=mu1_sb, in_=mu1_d[:, :])
            bias_sb = cpool.tile([128, KC], dt.float32)
            nc.sync.dma_start(out=bias_sb, in_=bias_d[:, :])
            seff_sb = cpool.tile([128, KT, NG], dt.float32)
            nc.sync.dma_start(
                out=seff_sb, in_=seff_d.rearrange("(t p) g -> p t g", p=128)
            )
            beff_sb = cpool.tile([128, KT, NG], dt.float32)
            nc.sync.dma_start(
                out=beff_sb, in_=beff_d.rearrange("(t p) g -> p t g", p=128)
            )

            # Resident transposed weight operand: [n % 128, n // 128, k]
            wt_sb = cpool.tile([128, NCH, KC], dt.bfloat16)

            # ---------------- Phase 1: dequant + transpose W ----------------
            with (
                tc.tile_pool(name="wq_pool", bufs=2) as wq_pool,
                tc.tile_pool(name="wstage", bufs=3) as ws_pool,
                tc.tile_pool(name="psw", bufs=2, space="PSUM") as psw_pool,
            ):
                for half in range(2):
                    wq_tiles = []
                    for i in range(4):
                        kt = half * 4 + i
                        wq_t = wq_pool.tile([128, N], dt.int32, name=f"wq_{i}")
                        nc.sync.dma_start(
                            out=wq_t, in_=wq_d[kt * 128 : (kt + 1) * 128, :]
                        )
                        wq_tiles.append((kt, wq_t))
                    for g in range(NG):
                        stage = ws_pool.tile([128, 4, 128], dt.bfloat16, name="wstg")
                        for i, (kt, wq_t) in enumerate(wq_tiles):
                            # (Q * s') + b'  with s' = scales*mu2, b' = -z*s*mu2
                            nc.vector.tensor_scalar(
                                out=stage[:, i, :],
                                in0=wq_t[:, g * 128 : (g + 1) * 128],
                                scalar1=seff_sb[:, kt, g : g + 1],
                                scalar2=beff_sb[:, kt, g : g + 1],
                                op0=mybir.AluOpType.mult,
                                op1=mybir.AluOpType.add,
                            )
                        ps = psw_pool.tile([128, 512], dt.bfloat16, name="psw")
                        for i in range(4):
                            nc.tensor.transpose(
                                ps[:, i * 128 : (i + 1) * 128], stage[:, i, :], ident
                            )
                        # evict with mu1[n] fold (per-partition scalar)
                        nc.vector.tensor_scalar_mul(
                            out=wt_sb[:, g, half * 512 : (half + 1) * 512],
                            in0=ps,
                            scalar1=mu1_sb[:, g : g + 1],
                        )

            # ---------------- Phase 2: stream x, matmul ----------------
            with (
                tc.tile_pool(name="xload", bufs=3) as xl_pool,
                tc.tile_pool(name="xt", bufs=2) as xt_pool,
                tc.tile_pool(name="psx", bufs=2, space="PSUM") as psx_pool,
                tc.tile_pool(name="pso", bufs=4, space="PSUM") as pso_pool,
                tc.tile_pool(name="osb", bufs=4) as osb_pool,
            ):
                def x_prep(mt, pool_tag=""):
                    xb = xl_pool.tile([128, N], dt.bfloat16, name="xb" + pool_tag)
                    if x_cast == "dma":
                        # SWDGE cast DMA: fp32 DRAM -> bf16 SBUF
                        nc.gpsimd.dma_start(
                            out=xb, in_=x_d[mt * 128 : (mt + 1) * 128, :]
                        )
                    else:
                        xf = xl_pool.tile([128, N], dt.float32, name="xf" + pool_tag)
                        nc.sync.dma_start(
                            out=xf, in_=x_d[mt * 128 : (mt + 1) * 128, :]
                        )
                        if x_cast == "act":
                            nc.scalar.copy(out=xb, in_=xf)
                        else:
                            nc.vector.tensor_copy(out=xb, in_=xf)
                    xt_t = xt_pool.tile([128, NCH, 128], dt.bfloat16, name="xt" + pool_tag)
                    if xT_dma:
                        # xbar DMA transpose SBUF->SBUF: [128 m, 4096 n] -> [n%128, n//128, m]
                        nc.scalar.dma_start(out=xt_t[:, :, :], in_=xb[:, :], transpose=True)
                    else:
                        for gb in range(NCH // 4):
                            ps = psx_pool.tile([128, 512], dt.bfloat16, name="psx")
                            for i in range(4):
                                g = gb * 4 + i
                                nc.tensor.transpose(
                                    ps[:, i * 128 : (i + 1) * 128],
                                    xb[:, g * 128 : (g + 1) * 128],
                                    ident,
                                )
                            nc.scalar.copy(
                                out=xt_t[:, gb * 4 : (gb + 1) * 4, :], in_=ps
                            )
                    return xt_t

                xt_fixed = x_prep(0, pool_tag="fix") if probe == "mm_only" else None
                for _rep in range(repeat):
                    for mt in range(MT):
                        if probe == "mm_only":
                            xt_t = xt_fixed
                        else:
                            xt_t = x_prep(mt)
                        if probe == "xprep_only":
                            continue
                        for kt2 in range(NKT):
                            pso = pso_pool.tile([128, KTILE], dt.float32, name="pso")
                            for g in range(NCH):
                                nc.tensor.matmul(
                                    pso,
                                    lhsT=xt_t[:, g, :],
                                    rhs=wt_sb[:, g, kt2 * KTILE : (kt2 + 1) * KTILE],
                                    start=(g == 0),
                                    stop=(g == NCH - 1),
                                )
                            osb = osb_pool.tile([128, KTILE], dt.float32, name="osb")
                            nc.vector.tensor_add(
                                out=osb,
                                in0=pso,
                                in1=bias_sb[:, kt2 * KTILE : (kt2 + 1) * KTILE],
                            )
                            nc.sync.dma_start(
                                out=out_d[
                                    mt * 128 : (mt + 1) * 128,
                                    kt2 * KTILE : (kt2 + 1) * KTILE,
                                ],
                                in_=osb,
                            )
    nc.compile()
    return nc


def make_in_maps(x, W_q, scales, zeros, mu1, mu2, bias):
    x2 = np.ascontiguousarray(np.asarray(x, dtype=np.float32).reshape(M, N))
    W_q = np.asarray(W_q, dtype=np.int32)
    scales = np.asarray(scales, dtype=np.float32).reshape(K, NG)
    zeros = np.asarray(zeros, dtype=np.float32).reshape(K, NG)
    mu1 = np.asarray(mu1, dtype=np.float32)
    mu2 = np.asarray(mu2, dtype=np.float32)
    bias = np.asarray(bias, dtype=np.float32)

    s_eff = scales * mu2[:, None]  # [K, NG]
    b_eff = -(zeros * s_eff)  # [K, NG]
    mu1_t = np.ascontiguousarray(mu1.reshape(NG, 128).T)  # [128, NG]

    in_maps = []
    for c in range(8):
        mi, ki = c // K_SH, c % K_SH
        in_maps.append(
            {
                "x": x2[mi * MC : (mi + 1) * MC],
                "wq": np.ascontiguousarray(W_q[ki * KC : (ki + 1) * KC]),
                "seff": np.ascontiguousarray(s_eff[ki * KC : (ki + 1) * KC]),
                "beff": np.ascontiguousarray(b_eff[ki * KC : (ki + 1) * KC]),
                "mu1t": mu1_t,
                "biasb": np.ascontiguousarray(
                    np.broadcast_to(bias[ki * KC : (ki + 1) * KC], (128, KC))
                ),
            }
        )
    return in_maps


def assemble(results):
    out = np.empty((M, K), np.float32)
    for c in range(8):
        mi, ki = c // K_SH, c % K_SH
        out[mi * MC : (mi + 1) * MC, ki * KC : (ki + 1) * KC] = results[c]["out"]
    return out.reshape(B, S, K)


def kernel(x, W_q, scales, zeros, mu1, mu2, bias):
    in_maps = make_in_maps(x, W_q, scales, zeros, mu1, mu2, bias)
    nc = _CACHE.get("nc")
    if nc is None:
        nc = build_nc()
        _CACHE["nc"] = nc
    res = run_bass_kernel_spmd(nc, in_maps, core_ids=list(range(8)))
    return assemble(res.results)



# revision 7
# speedup vs baseline: 1.2426x; 1.1369x over previous
"""Trainium2 Bass kernel for InvSGSS quantized linear.

out[m, k] = sum_n x[m, n] * W_deq[k, n] + bias[k]
W_deq[k, n] = (W_q[k, n] - zeros[k, g]) * scales[k, g] * mu2[k] * mu1[n],  g = n // 128

Sharding (8 cores): 2 m-shards x 4 k-shards. Each core handles
M_C=4096 rows of x and K_C=1024 output features.

Per-core dataflow:
  Phase 1 (once): stream W_q shard (uint8, host-packed) via SWDGE cast-DMA to
    bf16; dequant on DVE with fused tensor_scalar (W*s' + b') where
    s' = scales*mu2, b' = -zeros*scales*mu2 (host-folded small tensors);
    PE-transpose 128x128 chunks to build the resident W.T [n, k] bf16 operand,
    folding mu1[n] in during PSUM evict (ScalarE per-partition scale).
  Phase 2 (streamed): SWDGE cast-DMA x tiles [128m, N] fp32->bf16, xbar
    DMA-transpose to x.T [n-chunk, m] tiles, then accumulating bf16 matmuls
    per (m-tile, k-tile) psum; bias added on psum evict (DVE).
"""

import sys

if "/opt/trn_rl_repo" not in sys.path:
    sys.path.insert(0, "/opt/trn_rl_repo")

import numpy as np

import concourse.bass as bass  # noqa: F401
import concourse.mybir as mybir
import concourse.tile as tile
from concourse import bacc
from concourse.bass_utils import run_bass_kernel_spmd
from concourse.masks import make_identity

K, N = 4096, 4096
GROUP = 128
NG = N // GROUP  # 32 groups along N (group == 128-chunk)
M = 8192  # B*S
B, S = 4, 2048
M_SH, K_SH = 2, 4  # core grid: 2 m-shards x 4 k-shards
MC = M // M_SH  # 4096 rows per core
KC = K // K_SH  # 1024 output features per core
NCH = N // 128  # 32 contraction chunks
MT = MC // 128  # 32 m-tiles
KT = KC // 128  # 8 k-row-tiles of W
KTILE = 512  # matmul free dim (one PSUM bank)
NKT = KC // KTILE  # 2

_CACHE: dict = {}


def build_nc(
    repeat: int = 1,
    debug: bool = False,
    x_cast: str = "dma",
    probe: str = "full",
    xT_dma: bool = True,
    xT_eng: str = "scalar",
    xT_split: int = 1,
    w_mode: str = "u8",
    evict: str = "act",
    mm_pair: bool = True,
    osb_fuse: bool = True,
):
    """x_cast: 'dma' = SWDGE cast-DMA fp32->bf16; 'act' = HWDGE fp32 DMA + ScalarE cast;
    'vec' = HWDGE fp32 DMA + VectorE cast.
    probe: 'full' | 'mm_only' (skip x load/transpose in repeat body) |
    'xprep_only' (skip matmuls in repeat body).
    xT_dma: transpose x tiles via xbar DMA instead of the PE.
    xT_eng: HWDGE engine for the xbar transpose ('scalar' or 'sync').
    xT_split: number of xbar-transpose instructions per x tile.
    w_mode: 'u8cast' = uint8 W_q SWDGE cast-DMA -> bf16, DVE dequant bf16->bf16;
    'u8' = uint8 W_q HWDGE DMA, DVE dequant u8->bf16.
    evict: engine for the W-transpose PSUM evict + mu1 fold ('act' or 'vec').
    mm_pair: reuse each stationary x.T chunk for both k-halves back to back.
    osb_fuse: single [128, KC] output stage + one store per m-tile."""
    dt = mybir.dt
    nc = bacc.Bacc("TRN2", target_bir_lowering=False, debug=debug)

    x_d = nc.dram_tensor("x", [MC, N], dt.float32, kind="ExternalInput")
    wq_d = nc.dram_tensor("wq", [KC, N], dt.uint8, kind="ExternalInput")
    seff_d = nc.dram_tensor("seff", [KC, NG], dt.float32, kind="ExternalInput")
    beff_d = nc.dram_tensor("beff", [KC, NG], dt.float32, kind="ExternalInput")
    mu1_d = nc.dram_tensor("mu1t", [128, NG], dt.float32, kind="ExternalInput")
    bias_d = nc.dram_tensor("biasb", [128, KC], dt.float32, kind="ExternalInput")
    out_d = nc.dram_tensor("out", [MC, KC], dt.float32, kind="ExternalOutput")

    with tile.TileContext(nc) as tc:
        with tc.tile_pool(name="const", bufs=1) as cpool:
            ident = cpool.tile([128, 128], dt.bfloat16, tag="ident")
            make_identity(nc, ident)
            mu1_sb = cpool.tile([128, NG], dt.float32, tag="mu1")
            nc.sync.dma_start(out=mu1_sb, in_=mu1_d[:, :])
            bias_sb = cpool.tile([128, KC], dt.float32, tag="bias")
            nc.sync.dma_start(out=bias_sb, in_=bias_d[:, :])
            seff_sb = cpool.tile([128, KT, NG], dt.float32, tag="seff")
            nc.sync.dma_start(
                out=seff_sb, in_=seff_d.rearrange("(t p) g -> p t g", p=128)
            )
            beff_sb = cpool.tile([128, KT, NG], dt.float32, tag="beff")
            nc.sync.dma_start(
                out=beff_sb, in_=beff_d.rearrange("(t p) g -> p t g", p=128)
            )

            # Resident transposed weight operand: [n % 128, n // 128, k]
            wt_sb = cpool.tile([128, NCH, KC], dt.bfloat16, tag="wt")

            # ---------------- Phase 1: dequant + transpose W ----------------
            with (
                tc.tile_pool(name="wq_pool", bufs=2) as wq_pool,
                tc.tile_pool(name="wstage", bufs=3) as ws_pool,
                tc.tile_pool(name="psw", bufs=2, space="PSUM") as psw_pool,
            ):
                for half in range(2):
                    wq_tiles = []
                    for i in range(4):
                        kt = half * 4 + i
                        if w_mode == "u8cast":
                            wq_t = wq_pool.tile(
                                [128, N], dt.bfloat16, name=f"wq_{i}", tag=f"wq{i}"
                            )
                            nc.gpsimd.dma_start(
                                out=wq_t, in_=wq_d[kt * 128 : (kt + 1) * 128, :]
                            )
                        else:
                            wq_t = wq_pool.tile(
                                [128, N], dt.uint8, name=f"wq_{i}", tag=f"wq{i}"
                            )
                            nc.sync.dma_start(
                                out=wq_t, in_=wq_d[kt * 128 : (kt + 1) * 128, :]
                            )
                        wq_tiles.append((kt, wq_t))
                    for g in range(NG):
                        stage = ws_pool.tile([128, 4, 128], dt.bfloat16, name="wstg")
                        for i, (kt, wq_t) in enumerate(wq_tiles):
                            # (Q * s') + b'  with s' = scales*mu2, b' = -z*s*mu2
                            nc.vector.tensor_scalar(
                                out=stage[:, i, :],
                                in0=wq_t[:, g * 128 : (g + 1) * 128],
                                scalar1=seff_sb[:, kt, g : g + 1],
                                scalar2=beff_sb[:, kt, g : g + 1],
                                op0=mybir.AluOpType.mult,
                                op1=mybir.AluOpType.add,
                            )
                        ps = psw_pool.tile([128, 512], dt.bfloat16, name="psw")
                        for i in range(4):
                            nc.tensor.transpose(
                                ps[:, i * 128 : (i + 1) * 128], stage[:, i, :], ident
                            )
                        # evict with mu1[n] fold (per-partition scalar)
                        dst = wt_sb[:, g, half * 512 : (half + 1) * 512]
                        if evict == "act":
                            nc.scalar.mul(out=dst, in_=ps, mul=mu1_sb[:, g : g + 1])
                        else:
                            nc.vector.tensor_scalar_mul(
                                out=dst, in0=ps, scalar1=mu1_sb[:, g : g + 1]
                            )

            # ---------------- Phase 2: stream x, matmul ----------------
            with (
                tc.tile_pool(name="xload", bufs=3) as xl_pool,
                tc.tile_pool(name="xt", bufs=3) as xt_pool,
                tc.tile_pool(name="psx", bufs=2, space="PSUM") as psx_pool,
                tc.tile_pool(name="pso", bufs=2, space="PSUM") as pso_pool,
                tc.tile_pool(name="osb", bufs=3) as osb_pool,
            ):
                def x_load(mt, pool_tag=""):
                    xb = xl_pool.tile(
                        [128, N], dt.bfloat16, name="xb", tag="xb" + pool_tag
                    )
                    if x_cast == "dma":
                        # SWDGE cast DMA: fp32 DRAM -> bf16 SBUF
                        nc.gpsimd.dma_start(
                            out=xb, in_=x_d[mt * 128 : (mt + 1) * 128, :]
                        )
                    else:
                        xf = xl_pool.tile(
                            [128, N], dt.float32, name="xf", tag="xf" + pool_tag
                        )
                        nc.sync.dma_start(
                            out=xf, in_=x_d[mt * 128 : (mt + 1) * 128, :]
                        )
                        if x_cast == "act":
                            nc.scalar.copy(out=xb, in_=xf)
                        else:
                            nc.vector.tensor_copy(out=xb, in_=xf)
                    return xb

                def x_transpose(xb, pool_tag=""):
                    xt_t = xt_pool.tile(
                        [128, NCH, 128], dt.bfloat16, name="xt", tag="xt" + pool_tag
                    )
                    if xT_dma:
                        # xbar DMA transpose SBUF->SBUF:
                        # [128 m, 4096 n] -> [n%128, n//128, m]
                        eng = nc.scalar if xT_eng == "scalar" else nc.sync
                        w = NCH // xT_split
                        for sp in range(xT_split):
                            eng.dma_start(
                                out=xt_t[:, sp * w : (sp + 1) * w, :],
                                in_=xb[:, sp * w * 128 : (sp + 1) * w * 128],
                                transpose=True,
                            )
                    else:
                        for gb in range(NCH // 4):
                            ps = psx_pool.tile([128, 512], dt.bfloat16, name="psx")
                            for i in range(4):
                                g = gb * 4 + i
                                nc.tensor.transpose(
                                    ps[:, i * 128 : (i + 1) * 128],
                                    xb[:, g * 128 : (g + 1) * 128],
                                    ident,
                                )
                            nc.scalar.copy(
                                out=xt_t[:, gb * 4 : (gb + 1) * 4, :], in_=ps
                            )
                    return xt_t

                def x_prep(mt, pool_tag=""):
                    return x_transpose(x_load(mt, pool_tag), pool_tag)

                xt_fixed = x_prep(0, pool_tag="fix") if probe == "mm_only" else None
                xb_fixed = x_load(0, pool_tag="fix") if probe == "xt_only" else None
                for _rep in range(repeat):
                    for mt in range(MT):
                        if probe == "xload_only":
                            x_load(mt)
                            continue
                        if probe == "xt_only":
                            x_transpose(xb_fixed)
                            continue
                        if probe == "mm_only":
                            xt_t = xt_fixed
                        else:
                            xt_t = x_prep(mt)
                        if probe == "xprep_only":
                            continue
                        psos = [
                            pso_pool.tile(
                                [128, KTILE], dt.float32, name=f"pso{j}", tag=f"pso{j}"
                            )
                            for j in range(NKT)
                        ]
                        if mm_pair:
                            for g in range(NCH):
                                for j in range(NKT):
                                    nc.tensor.matmul(
                                        psos[j],
                                        lhsT=xt_t[:, g, :],
                                        rhs=wt_sb[
                                            :, g, j * KTILE : (j + 1) * KTILE
                                        ],
                                        start=(g == 0),
                                        stop=(g == NCH - 1),
                                    )
                        else:
                            for j in range(NKT):
                                for g in range(NCH):
                                    nc.tensor.matmul(
                                        psos[j],
                                        lhsT=xt_t[:, g, :],
                                        rhs=wt_sb[
                                            :, g, j * KTILE : (j + 1) * KTILE
                                        ],
                                        start=(g == 0),
                                        stop=(g == NCH - 1),
                                    )
                        if osb_fuse:
                            osb = osb_pool.tile([128, KC], dt.float32, name="osb")
                            for j in range(NKT):
                                nc.vector.tensor_add(
                                    out=osb[:, j * KTILE : (j + 1) * KTILE],
                                    in0=psos[j],
                                    in1=bias_sb[:, j * KTILE : (j + 1) * KTILE],
                                )
                            nc.sync.dma_start(
                                out=out_d[mt * 128 : (mt + 1) * 128, :], in_=osb
                            )
                        else:
                            for j in range(NKT):
                                osb = osb_pool.tile(
                                    [128, KTILE], dt.float32, name="osb"
                                )
                                nc.vector.tensor_add(
                                    out=osb,
                                    in0=psos[j],
                                    in1=bias_sb[:, j * KTILE : (j + 1) * KTILE],
                                )
                                nc.sync.dma_start(
                                    out=out_d[
                                        mt * 128 : (mt + 1) * 128,
                                        j * KTILE : (j + 1) * KTILE,
                                    ],
                                    in_=osb,
                                )
    nc.compile()
    return nc


def make_in_maps(x, W_q, scales, zeros, mu1, mu2, bias):
    x2 = np.ascontiguousarray(np.asarray(x, dtype=np.float32).reshape(M, N))
    W_q = np.asarray(W_q).astype(np.uint8)
    scales = np.asarray(scales, dtype=np.float32).reshape(K, NG)
    zeros = np.asarray(zeros, dtype=np.float32).reshape(K, NG)
    mu1 = np.asarray(mu1, dtype=np.float32)
    mu2 = np.asarray(mu2, dtype=np.float32)
    bias = np.asarray(bias, dtype=np.float32)

    s_eff = scales * mu2[:, None]  # [K, NG]
    b_eff = -(zeros * s_eff)  # [K, NG]
    mu1_t = np.ascontiguousarray(mu1.reshape(NG, 128).T)  # [128, NG]

    in_maps = []
    for c in range(8):
        mi, ki = c // K_SH, c % K_SH
        in_maps.append(
            {
                "x": x2[mi * MC : (mi + 1) * MC],
                "wq": np.ascontiguousarray(W_q[ki * KC : (ki + 1) * KC]),
                "seff": np.ascontiguousarray(s_eff[ki * KC : (ki + 1) * KC]),
                "beff": np.ascontiguousarray(b_eff[ki * KC : (ki + 1) * KC]),
                "mu1t": mu1_t,
                "biasb": np.ascontiguousarray(
                    np.broadcast_to(bias[ki * KC : (ki + 1) * KC], (128, KC))
                ),
            }
        )
    return in_maps


def assemble(results):
    out = np.empty((M, K), np.float32)
    for c in range(8):
        mi, ki = c // K_SH, c % K_SH
        out[mi * MC : (mi + 1) * MC, ki * KC : (ki + 1) * KC] = results[c]["out"]
    return out.reshape(B, S, K)


def kernel(x, W_q, scales, zeros, mu1, mu2, bias):
    in_maps = make_in_maps(x, W_q, scales, zeros, mu1, mu2, bias)
    nc = _CACHE.get("nc")
    if nc is None:
        nc = build_nc()
        _CACHE["nc"] = nc
    res = run_bass_kernel_spmd(nc, in_maps, core_ids=list(range(8)))
    return assemble(res.results)
